# revision 11
# baseline (speedup 1.0000x reference)
"""Trainium2 Bass kernel for CoarseMatching (retrieval kNN, L=S=6400, C=256).

Strategy (8 NeuronCores, SPMD — one Bass module, per-core input data):
  - Host transposes features to [C, L] once ("layout glue").
  - Core c owns query-block rows [800c, 800c+800) of the distance matrix for
    BOTH directions (forward: feat0 block vs all feat1; reverse: feat1 block
    vs all feat0), so no cross-core collectives are needed.
  - On device, per direction:  v[q, s] = dot(fq, fs) - |fs|^2 / 2   so that
    argmax_s v  ==  argmin_s d, with d_raw = -2*(negh[q] + v) and
    negh[x] = -|fx|^2/2 computed on device by PE matmuls over squared
    features (squares on GPSIMD).  The -|fs|^2/2 term is folded into the
    matmul as an augmented K=1 rank-1 update (ones (x) negh_row).
  - PSUM->SBUF evacuation on the scalar engine; exact top-8 values+indices
    per row via DVE max / max_index (tie-break = lowest index, matching
    jax.lax.top_k).
  - Host reconstructs the top-3 squared distances from (negh, v), applies
    sqrt, and runs the tiny O(L*5) mutual-NN / ratio / geometric epilogue.
"""

import numpy as np

L = 6400
C = 256
NCORES = 8
B = L // NCORES  # 800 rows per core per direction
L_SIDE = 80
NUM_SAMPLES = 5
RATIO_THR = np.float32(0.85)
GEOM_THR = np.float32(0.1)
ANG_PAD = np.float32(0.1)
COS_CLIP = np.float32(0.99)

# 13 N-pieces of the 6400-wide moving dimension (PSUM bank = 512 fp32)
PIECES = [(i * 512, 512) for i in range(12)] + [(6144, 256)]
# 7 M-subtiles of the 800-row block
MTILES = [(j * 128, 128) for j in range(6)] + [(768, 32)]

_MODULE_CACHE = {}


def _build_module():
    """Build the SPMD Bass module (identical for all cores)."""
    from contextlib import ExitStack

    import concourse.bacc as bacc
    import concourse.mybir as mybir
    import concourse.tile as tile

    f32 = mybir.dt.float32
    u32 = mybir.dt.uint32
    COPY = mybir.ActivationFunctionType.Copy
    SQUARE = mybir.ActivationFunctionType.Square

    nc = bacc.Bacc(trn_type="TRN2")

    # DRAM I/O.  r* are the full transposed features, pre-chunked by K on the
    # host: [chunk, 128, L].  q* are this core's own column-block of the same
    # arrays: [chunk, 128, B].
    r_dram = [
        nc.dram_tensor("r0", (2, 128, L), f32, kind="ExternalInput"),
        nc.dram_tensor("r1", (2, 128, L), f32, kind="ExternalInput"),
    ]
    q_dram = [
        nc.dram_tensor("q0", (2, 128, B), f32, kind="ExternalInput"),
        nc.dram_tensor("q1", (2, 128, B), f32, kind="ExternalInput"),
    ]
    vals_out = nc.dram_tensor("vals", (2, B, 8), f32, kind="ExternalOutput")
    idx_out = nc.dram_tensor("idx", (2, B, 8), u32, kind="ExternalOutput")
    negh_out = nc.dram_tensor("negh", (2, 13, 512), f32, kind="ExternalOutput")

    with tile.TileContext(nc) as tc, ExitStack() as ctx:
        const_pool = ctx.enter_context(tc.tile_pool(name="const", bufs=1))
        rpool = ctx.enter_context(tc.tile_pool(name="r", bufs=1))
        qpool = ctx.enter_context(tc.tile_pool(name="q", bufs=1))
        sqpool = ctx.enter_context(tc.tile_pool(name="sq", bufs=3))
        vpool = ctx.enter_context(tc.tile_pool(name="v", bufs=2))
        s8pool = ctx.enter_context(tc.tile_pool(name="s8", bufs=4))
        pmain = ctx.enter_context(tc.tile_pool(name="pmain", bufs=6, space="PSUM"))
        pnorm = ctx.enter_context(tc.tile_pool(name="pnorm", bufs=2, space="PSUM"))

        # Constants
        neg_half = const_pool.tile([128, 1], f32, tag="neghalf")
        nc.gpsimd.memset(neg_half[:], -0.5)
        ones_aug = const_pool.tile([128, 128], f32, tag="ones_aug")
        nc.gpsimd.memset(ones_aug[:], 1.0)

        # Load full transposed features as per-piece tiles (one DMA writer per
        # tile keeps the sync-wait count per consumer instruction small).
        rt = {}
        for ti in (0, 1):
            for ch in range(2):
                for pi, (off, w) in enumerate(PIECES):
                    t = rpool.tile([128, w], f32, tag=f"r{ti}{ch}p{pi}", name=f"r{ti}{ch}p{pi}")
                    rt[ti, ch, pi] = t
                    nc.sync.dma_start(t[:], r_dram[ti][ch, :, off : off + w])
        qt = {}
        for ti in (0, 1):
            for ch in range(2):
                t = qpool.tile([128, B], f32, tag=f"q{ti}{ch}")
                qt[ti, ch] = t
                nc.sync.dma_start(t[:], q_dram[ti][ch])

        # negh[ti]: piece p lives at partition base (p%4)*32, byte-column
        # (p//3)*512 of a [128, 2560] tile, so each piece can serve as the
        # K=1 augmented-matmul rhs (PE base partitions must be 0/32/64).
        def npart(p):
            return (p % 3) * 32

        def ncol(p):
            return (p // 3) * 512

        negh = {}
        for ti in (0, 1):
            nt = const_pool.tile([128, 2560], f32, tag=f"negh{ti}")
            negh[ti] = nt
            for pi, (off, w) in enumerate(PIECES):
                bp, bc = npart(pi), ncol(pi)
                ps = pnorm.tile([128, 512], f32, tag="pnorm")
                for ch in range(2):
                    sq = sqpool.tile([128, 512], f32, tag="sq")
                    nc.scalar.activation(sq[:, :w], rt[ti, ch, pi][:], SQUARE)
                    nc.tensor.matmul(
                        ps[bp : bp + 1, :w],
                        neg_half[:],
                        sq[:, :w],
                        start=(ch == 0),
                        stop=(ch == 1),
                    )
                nc.scalar.activation(
                    nt[bp : bp + 1, bc : bc + w], ps[bp : bp + 1, :w], COPY
                )
                nc.sync.dma_start(
                    negh_out[ti, pi : pi + 1, :w], nt[bp : bp + 1, bc : bc + w]
                )

        # Main loop: dir 0 = forward (f0 block vs all f1), dir 1 = reverse.
        for dir_ in (0, 1):
            qc = (qt[dir_, 0], qt[dir_, 1])
            ngh = negh[1 - dir_]
            for moff, msz in MTILES:
                v = vpool.tile([128, L], f32, tag="v")
                for pg in range(0, 13, 4):
                    pcs = list(range(pg, min(pg + 4, 13)))
                    pstiles = {
                        p: pmain.tile([128, 512], f32, tag="ps", name=f"ps{p}")
                        for p in pcs
                    }
                    # chunk-major within the group => stationary reuse runs
                    for p in pcs:
                        off, w = PIECES[p]
                        nc.tensor.matmul(
                            pstiles[p][:msz, :w],
                            qc[0][:, moff : moff + msz],
                            rt[1 - dir_, 0, p][:],
                            start=True,
                            stop=False,
                        )
                    for p in pcs:
                        off, w = PIECES[p]
                        nc.tensor.matmul(
                            pstiles[p][:msz, :w],
                            qc[1][:, moff : moff + msz],
                            rt[1 - dir_, 1, p][:],
                            start=False,
                            stop=False,
                        )
                    for p in pcs:
                        off, w = PIECES[p]
                        bp, bc = npart(p), ncol(p)
                        nc.tensor.matmul(
                            pstiles[p][:msz, :w],
                            ones_aug[bp : bp + 1, :msz],
                            ngh[bp : bp + 1, bc : bc + w],
                            start=False,
                            stop=True,
                        )
                    for p in pcs:
                        off, w = PIECES[p]
                        nc.scalar.activation(
                            v[:msz, off : off + w], pstiles[p][:msz, :w], COPY
                        )
                v8 = s8pool.tile([128, 8], f32, tag="v8")
                i8 = s8pool.tile([128, 8], u32, tag="i8")
                nc.vector.max(v8[:msz], v[:msz])
                nc.vector.max_index(i8[:msz], v8[:msz], v[:msz])
                nc.sync.dma_start(vals_out[dir_, moff : moff + msz, :], v8[:msz])
                nc.sync.dma_start(idx_out[dir_, moff : moff + msz, :], i8[:msz])

    nc.compile()
    return nc


def _get_module():
    if "nc" not in _MODULE_CACHE:
        _MODULE_CACHE["nc"] = _build_module()
    return _MODULE_CACHE["nc"]


def _make_in_maps(f0T, f1T):
    r0 = np.ascontiguousarray(f0T.reshape(2, 128, L))
    r1 = np.ascontiguousarray(f1T.reshape(2, 128, L))
    in_maps = []
    for c in range(NCORES):
        q0 = np.ascontiguousarray(f0T[:, c * B : (c + 1) * B].reshape(2, 128, B))
        q1 = np.ascontiguousarray(f1T[:, c * B : (c + 1) * B].reshape(2, 128, B))
        in_maps.append({"r0": r0, "r1": r1, "q0": q0, "q1": q1})
    return in_maps


def _run_device(f0T, f1T):
    """Run the 8-core SPMD kernel; returns (vals [2,L,8], idx [2,L,8], negh [2,L])."""
    from concourse.bass_utils import run_bass_kernel_spmd

    nc = _get_module()
    in_maps = _make_in_maps(f0T, f1T)
    res = run_bass_kernel_spmd(nc, in_maps, core_ids=list(range(NCORES))).results
    vals = np.concatenate([res[c]["vals"] for c in range(NCORES)], axis=1)
    idx = np.concatenate([res[c]["idx"] for c in range(NCORES)], axis=1)
    negh = res[0]["negh"].reshape(2, -1)[:, :L]
    return vals, idx, negh


def _emulate_device(f0T, f1T):
    """Numpy emulation of the device computation (same math, for testing)."""
    f0 = f0T.T.astype(np.float32)
    f1 = f1T.T.astype(np.float32)
    negh = np.stack(
        [-0.5 * np.sum(f0 * f0, 1), -0.5 * np.sum(f1 * f1, 1)]
    ).astype(np.float32)
    vals = np.zeros((2, L, 8), np.float32)
    idx = np.zeros((2, L, 8), np.uint32)
    for dir_ in (0, 1):
        a, b = (f0, f1) if dir_ == 0 else (f1, f0)
        v = (a @ b.T + negh[1 - dir_][None, :]).astype(np.float32)
        order = np.argsort(-v, axis=1, kind="stable")[:, :8]
        vals[dir_] = np.take_along_axis(v, order, axis=1)
        idx[dir_] = order.astype(np.uint32)
    return vals, idx, negh


def _epilogue(vals, idx, negh, sample_idx):
    """Host epilogue mirroring reference.py in float32."""
    f32 = np.float32
    # top-3 squared distances (scaled by 1/C):  d = -2*(negh_self + v) / C
    negh_self0 = negh[0]  # -|f0_q|^2/2 for forward rows
    negh_self1 = negh[1]
    d1 = (-(negh_self0[:, None] + vals[0, :, :3]) / f32(128.0)).astype(f32)
    d2 = (-(negh_self1[:, None] + vals[1, :, :3]) / f32(128.0)).astype(f32)
    preds1 = idx[0, :, :3].astype(np.int32)
    preds2 = idx[1, :, :3].astype(np.int32)

    # border mask
    m = np.ones((L_SIDE, L_SIDE), dtype=bool)
    m[:2, :] = False
    m[:, :2] = False
    m[-2:, :] = False
    m[:, -2:] = False
    bmask = m.reshape(-1)

    keep1 = (d1[:, 0] / d1[:, 1] <= RATIO_THR) & bmask
    keep2 = (d2[:, 0] / d2[:, 1] <= RATIO_THR) & bmask

    j0 = preds1[:, 0]
    ar = np.arange(L, dtype=np.int64)
    mutual = keep1 & keep2[j0] & (preds2[j0, 0] == ar)

    x1 = (ar % L_SIDE).astype(f32)
    y1 = (ar // L_SIDE).astype(f32)
    x2 = (j0 % L_SIDE).astype(f32)
    y2 = (j0 // L_SIDE).astype(f32)

    sidx = np.asarray(sample_idx).astype(np.int64)
    d1s = np.hypot(x1[:, None] - x1[sidx], y1[:, None] - y1[sidx]).astype(f32)
    d2s = np.hypot(x2[:, None] - x2[sidx], y2[:, None] - y2[sidx]).astype(f32)

    b_idx = sidx[:, [4, 1, 2, 3, 4]]
    c_idx = sidx[:, [0, 0, 1, 2, 3]]

    def theta(ax, ay, bx, by, cx, cy):
        abx, aby = ax - bx, ay - by
        acx, acy = ax - cx, ay - cy
        denom = np.maximum(
            np.hypot(abx, aby) * np.hypot(acx, acy), f32(1e-8)
        ).astype(f32)
        cos = ((abx * acx + aby * acy) / denom).astype(f32)
        return np.arccos(np.clip(cos, -COS_CLIP, COS_CLIP)).astype(f32)

    theta1 = theta(x1[:, None], y1[:, None], x1[b_idx], y1[b_idx], x1[c_idx], y1[c_idx])
    theta2 = theta(x2[:, None], y2[:, None], x2[b_idx], y2[b_idx], x2[c_idx], y2[c_idx])
    ang1 = np.degrees(theta1 + ANG_PAD).astype(f32)
    ang2 = np.degrees(theta2 + ANG_PAD).astype(f32)

    ratio = (d1s / np.maximum(d2s, f32(1e-6))).astype(f32)
    mN = f32(max(int(mutual.sum()), 1))
    dist_center = f32(
        np.sum(np.where(mutual[:, None], ratio, f32(0.0)), dtype=np.float32)
        / (mN * f32(NUM_SAMPLES))
    )
    dist_means = np.mean(np.abs(ratio / dist_center - f32(1.0)), axis=1, dtype=np.float32)
    angle_means = np.mean(np.abs(ang1 / ang2 - f32(1.0)), axis=1, dtype=np.float32)

    geom_keep = mutual & (dist_means < GEOM_THR) & (angle_means < GEOM_THR)

    return (
        np.sqrt(d1).astype(f32),
        preds1,
        np.sqrt(d2).astype(f32),
        preds2,
        mutual,
        dist_means.astype(f32),
        angle_means.astype(f32),
        geom_keep,
    )


def kernel(feat_c0, feat_c1, sample_idx, _emulate=False):
    f0T = np.ascontiguousarray(np.asarray(feat_c0, np.float32)[0].T)  # [C, L]
    f1T = np.ascontiguousarray(np.asarray(feat_c1, np.float32)[0].T)
    if _emulate:
        vals, idx, negh = _emulate_device(f0T, f1T)
    else:
        vals, idx, negh = _run_device(f0T, f1T)
    return _epilogue(vals, idx, negh, sample_idx)
